# revision 1
# baseline (speedup 1.0000x reference)
"""Conv1D + 2x LSTM(relu) + dense/softmax actor model on 8 Trainium2 cores.

Strategy: pure data parallel over batch (128 -> 16 per core); params
replicated. Everything kept on-chip in a "transposed" layout
([units on partitions, batch on free]) so the sequential LSTM recurrence
never needs an on-chip transpose:

  - conv expressed as a K=2 matmul producing xT [64, batch, time] (bf16)
  - per step, gate pre-activations z_gT [100, batch] are built in PSUM:
    the input-side contributions (W1 @ x_t, W2 @ h1_t, biases via an
    augmented ones-row) are batched 8 timesteps per matmul, and the
    recurrent parts (U @ h_{t-1}) accumulate on top with the weight
    matrix as the PE-stationary operand (bf16, M padded to 128 for FWL).
  - gates are stored in [i, f, o, g] order so one ACT sigmoid covers
    i/f/o; relu(g) is folded into DVE scalar_tensor_tensor ops.
  - cell state c kept fp32; h written directly as bf16 for the matmuls.
"""

import numpy as np

import concourse.bass as bass
import concourse.bacc as bacc
import concourse.mybir as mybir
import concourse.tile as tile
from concourse.bass_utils import run_bass_kernel_spmd

# Problem constants (hardcoded: harness runs kernel.py standalone).
B = 128          # batch
T = 2048         # input sequence length
A = 3            # actions
H = 100          # LSTM units
F = 64           # conv filters
NCORES = 8
BS = B // NCORES  # 16 batch rows per core

GN = 4            # gates
GP = 128          # padded gate size (full 128-col stationary => FWL)
BLK = 8           # timestep block for batched input-side matmuls
RING = 2 * BLK    # h1 ring buffer slots
CCH = 32          # conv time-chunk (N = BS*CCH = 512)
# our gate order [i, f, o, g]; reference weight layout is [i, f, g, o]
GMAP = (0, 1, 3, 2)

f32 = mybir.dt.float32
bf16 = mybir.dt.bfloat16
FT = mybir.ActivationFunctionType
OP = mybir.AluOpType


def build_bass(seq_len=T):
    """Build the single-core program (SPMD: same NEFF on all 8 cores)."""
    TS = seq_len - 1  # conv(kernel=2, VALID) output length
    nc = bacc.Bacc(
        "TRN2",
        target_bir_lowering=False,
        debug=False,
        num_devices=NCORES,
    )

    st_d = nc.dram_tensor("state_input", [BS, seq_len], f32, kind="ExternalInput")
    cw_d = nc.dram_tensor("conv_w", [2, 1, F], f32, kind="ExternalInput")
    cb_d = nc.dram_tensor("conv_b", [F], f32, kind="ExternalInput")
    w1_d = nc.dram_tensor("lstm1_w", [F, GN * H], f32, kind="ExternalInput")
    u1_d = nc.dram_tensor("lstm1_u", [H, GN * H], f32, kind="ExternalInput")
    b1_d = nc.dram_tensor("lstm1_b", [GN * H], f32, kind="ExternalInput")
    w2_d = nc.dram_tensor("lstm2_w", [H, GN * H], f32, kind="ExternalInput")
    u2_d = nc.dram_tensor("lstm2_u", [H, GN * H], f32, kind="ExternalInput")
    b2_d = nc.dram_tensor("lstm2_b", [GN * H], f32, kind="ExternalInput")
    dw_d = nc.dram_tensor("dense_w", [H, A], f32, kind="ExternalInput")
    db_d = nc.dram_tensor("dense_b", [A], f32, kind="ExternalInput")
    out_d = nc.dram_tensor("out", [BS, A], f32, kind="ExternalOutput")

    with tile.TileContext(nc) as tc:
        with (
            tc.tile_pool(name="const", bufs=1) as const,
            tc.tile_pool(name="prep", bufs=2) as prep,
            tc.tile_pool(name="sig", bufs=4) as sigp,
            tc.tile_pool(name="tmp", bufs=4) as tmpp,
            tc.tile_pool(name="z1pool", bufs=2, space="PSUM") as z1pool,
            tc.tile_pool(name="z2pool", bufs=2, space="PSUM") as z2pool,
            tc.tile_pool(name="convpool", bufs=2, space="PSUM") as convpool,
            tc.tile_pool(name="miscpsum", bufs=1, space="PSUM") as miscpsum,
        ):
            # ---------------- input staging ----------------
            s_f32 = prep.tile([BS, seq_len], f32)
            nc.sync.dma_start(out=s_f32, in_=st_d[:, :])
            s_bf = prep.tile([BS, seq_len], bf16)
            nc.vector.tensor_copy(out=s_bf, in_=s_f32)
            # S2[k, b, t] = s[b, t+k]  (conv rhs, contraction dim K=2)
            S2 = const.tile([2, BS, TS], bf16)
            for k in range(2):
                nc.sync.dma_start(out=S2[k : k + 1, :, :], in_=s_bf[:, k : k + TS])

            # xT augmented with a ones-row (bias via matmul)
            xTa = const.tile([F + 1, BS, TS], bf16)
            nc.vector.memset(xTa[F : F + 1, :, :], 1.0)
            # h1 ring, augmented ones-row for W2's bias. Partition ranges
            # must start 32-aligned, so memset [96:101]; rows 96-99 are
            # rewritten with real h values before any consumer reads them.
            ring = const.tile([H + 1, RING, BS], bf16)
            nc.vector.memset(ring[96 : H + 1, :, :], 1.0)

            # ---------------- weights ----------------
            def load_wu(w_dram, b_dram, K, name):
                P = K + (1 if b_dram is not None else 0)
                stage = prep.tile([P, GN * H], f32, tag=f"wstage_{name}")
                if b_dram is not None:
                    # bias row lives at partition K; partition starts must be
                    # 32-aligned, so broadcast into [aligned:K+1] first and
                    # let the weight DMA below overwrite rows [aligned:K).
                    al = (K // 32) * 32
                    bias_bcast = bass.AP(
                        tensor=b_dram[:].tensor,
                        offset=0,
                        ap=[[0, K + 1 - al], [1, GN * H]],
                    )
                    nc.gpsimd.dma_start(out=stage[al : K + 1, :], in_=bias_bcast)
                nc.sync.dma_start(out=stage[0:K, :], in_=w_dram[:, :])
                wt = const.tile([P, GN, GP], bf16, tag=f"wt_{name}")
                for g in range(GN):
                    rg = GMAP[g]
                    nc.vector.tensor_copy(
                        out=wt[:, g, 0:H], in_=stage[:, rg * H : (rg + 1) * H]
                    )
                    nc.vector.memset(wt[:, g, H:GP], 0.0)
                return wt

            U1 = load_wu(u1_d, None, H, "u1")     # [100, 4, 128]
            U2 = load_wu(u2_d, None, H, "u2")     # [100, 4, 128]
            W1b = load_wu(w1_d, b1_d, F, "w1")    # [65, 4, 128]
            W2b = load_wu(w2_d, b2_d, H, "w2")    # [101, 4, 128]

            cwstage = prep.tile([2, F], f32)
            nc.sync.dma_start(out=cwstage, in_=cw_d[:, 0, :])
            cw_bf = const.tile([2, F], bf16)
            nc.vector.tensor_copy(out=cw_bf, in_=cwstage)
            cb_sb = const.tile([F, 1], f32)
            nc.sync.dma_start(out=cb_sb, in_=cb_d[:])

            dw_sb = const.tile([H, A], f32)
            nc.sync.dma_start(out=dw_sb, in_=dw_d[:, :])
            db_sb = const.tile([BS, A], f32)
            db_bcast = bass.AP(
                tensor=db_d[:].tensor, offset=0, ap=[[0, BS], [1, A]]
            )
            nc.gpsimd.dma_start(out=db_sb, in_=db_bcast)

            # ---------------- conv as K=2 matmul ----------------
            for c0 in range(0, TS, CCH):
                cn = min(CCH, TS - c0)
                cp = convpool.tile([F, BS, CCH], f32, tag="convp")
                nc.tensor.matmul(
                    out=cp[:, :, 0:cn],
                    lhsT=cw_bf,
                    rhs=S2[:, :, c0 : c0 + cn],
                    start=True,
                    stop=True,
                )
                nc.scalar.activation(
                    out=xTa[0:F, :, c0 : c0 + cn],
                    in_=cp[:, :, 0:cn],
                    func=FT.Relu,
                    bias=cb_sb,
                    scale=1.0,
                )

            # ---------------- the scan ----------------
            c1 = const.tile([H, BS], f32)
            c2 = const.tile([H, BS], f32)
            h2 = const.tile([H, BS], bf16)
            h2f = const.tile([H, BS], f32)

            state = {"z1": None, "z2": None}

            def cell(s, which):
                """Emit one LSTM step. which=1: reads xTa, writes ring.
                which=2: reads ring, writes h2 (h2f on the last step)."""
                if which == 1:
                    zpool, Wb, U, cc = z1pool, W1b, U1, c1
                else:
                    zpool, Wb, U, cc = z2pool, W2b, U2, c2
                zkey = "z%d" % which
                bi = s % BLK
                if bi == 0:
                    zc = zpool.tile([GP, GN, BS, BLK], f32, tag=zkey)
                    state[zkey] = zc
                    n = min(BLK, TS - s)
                    if which == 1:
                        rhs = xTa[:, :, s : s + n]
                    else:
                        base = (s // BLK) % 2 * BLK
                        rhs = ring[:, base : base + n, :].rearrange(
                            "p s b -> p b s"
                        )
                    for g in range(GN):
                        nc.tensor.matmul(
                            out=zc[:, g, :, 0:n],
                            lhsT=Wb[:, g, :],
                            rhs=rhs,
                            start=True,
                            stop=False,
                            skip_group_check=True,
                        )
                zc = state[zkey]
                if s > 0:
                    rhs = ring[0:H, (s - 1) % RING, :] if which == 1 else h2
                    for g in range(GN):
                        nc.tensor.matmul(
                            out=zc[:, g, :, bi],
                            lhsT=U[:, g, :],
                            rhs=rhs,
                            start=False,
                            stop=True,
                            skip_group_check=True,
                        )
                sg = sigp.tile([H, 3, BS], f32, tag="sg%d" % which)
                nc.scalar.activation(
                    out=sg, in_=zc[0:H, 0:3, :, bi], func=FT.Sigmoid
                )
                zg = zc[0:H, 3, :, bi]
                if which == 1:
                    hout = ring[0:H, s % RING, :]
                elif s == TS - 1:
                    hout = h2f
                else:
                    hout = h2
                if s == 0:
                    # c = i * relu(g)
                    nc.vector.scalar_tensor_tensor(
                        out=cc, in0=zg, scalar=0.0, in1=sg[:, 0, :],
                        op0=OP.max, op1=OP.mult,
                    )
                else:
                    t1 = tmpp.tile([H, BS], f32, tag="t1_%d" % which)
                    nc.vector.scalar_tensor_tensor(
                        out=t1, in0=zg, scalar=0.0, in1=sg[:, 0, :],
                        op0=OP.max, op1=OP.mult,
                    )
                    t2 = tmpp.tile([H, BS], f32, tag="t2_%d" % which)
                    nc.vector.tensor_mul(out=t2, in0=sg[:, 1, :], in1=cc)
                    nc.vector.tensor_add(out=cc, in0=t1, in1=t2)
                # h = o * relu(c)
                nc.vector.scalar_tensor_tensor(
                    out=hout, in0=cc, scalar=0.0, in1=sg[:, 2, :],
                    op0=OP.max, op1=OP.mult,
                )

            for s in range(TS):
                cell(s, 1)
                u = s - BLK
                if u >= 0:
                    cell(u, 2)
            for u in range(max(TS - BLK, 0), TS):
                cell(u, 2)

            # ---------------- dense + softmax ----------------
            lg_ps = miscpsum.tile([BS, A], f32)
            nc.tensor.matmul(
                out=lg_ps, lhsT=h2f, rhs=dw_sb, start=True, stop=True
            )
            lg = tmpp.tile([BS, A], f32, tag="lg")
            nc.vector.tensor_add(out=lg, in0=lg_ps, in1=db_sb)
            mx = tmpp.tile([BS, 1], f32, tag="mx")
            nc.vector.tensor_reduce(
                out=mx, in_=lg, axis=mybir.AxisListType.X, op=OP.max
            )
            nmx = tmpp.tile([BS, 1], f32, tag="nmx")
            nc.vector.tensor_scalar_mul(out=nmx, in0=mx, scalar1=-1.0)
            ex = tmpp.tile([BS, A], f32, tag="ex")
            nc.scalar.activation(out=ex, in_=lg, func=FT.Exp, bias=nmx, scale=1.0)
            sm = tmpp.tile([BS, 1], f32, tag="sm")
            nc.vector.tensor_reduce(
                out=sm, in_=ex, axis=mybir.AxisListType.X, op=OP.add
            )
            rc = tmpp.tile([BS, 1], f32, tag="rc")
            nc.vector.reciprocal(out=rc, in_=sm)
            ot = tmpp.tile([BS, A], f32, tag="ot")
            nc.vector.tensor_scalar_mul(out=ot, in0=ex, scalar1=rc)
            nc.sync.dma_start(out=out_d[:, :], in_=ot)

    nc.finalize()
    return nc


_NC_CACHE = {}


def _get_nc(seq_len=T):
    if seq_len not in _NC_CACHE:
        _NC_CACHE[seq_len] = build_bass(seq_len)
    return _NC_CACHE[seq_len]


def kernel(**inputs):
    return run(inputs, seq_len=T, trace=False)[0]


def run(inputs, seq_len=T, trace=False):
    """Returns (full_output [B, A] f32, BassKernelResults)."""
    nc = _get_nc(seq_len)
    state = np.ascontiguousarray(
        np.asarray(inputs["state_input"], dtype=np.float32).reshape(B, seq_len)
    )
    shared = {}
    for k in (
        "conv_w", "conv_b", "lstm1_w", "lstm1_u", "lstm1_b",
        "lstm2_w", "lstm2_u", "lstm2_b", "dense_w", "dense_b",
    ):
        shared[k] = np.ascontiguousarray(np.asarray(inputs[k], dtype=np.float32))
    in_maps = []
    for c in range(NCORES):
        m = dict(shared)
        m["state_input"] = np.ascontiguousarray(state[c * BS : (c + 1) * BS])
        in_maps.append(m)
    res = run_bass_kernel_spmd(
        nc, in_maps, core_ids=list(range(NCORES)), trace=trace
    )
    out = np.concatenate([r["out"] for r in res.results], axis=0)
    return out.astype(np.float32), res



# revision 3
# speedup vs baseline: 1.2177x; 1.2177x over previous
"""Conv1D + 2x LSTM(relu) + dense/softmax actor model on 8 Trainium2 cores.

Strategy: pure data parallel over batch (128 -> 16 per core); params
replicated. Everything kept on-chip in a "transposed" layout
([units on partitions, batch on free]) so the sequential LSTM recurrence
never needs an on-chip transpose.

v2: the two LSTM cells are processed in lock-step (cell2 lags cell1 by
BLK steps) with their gate pre-activations in ONE shared PSUM tile
[128, 2cells, 4gates, BS, BLK], so each step needs a single sigmoid ACT
(i,f,o for both cells) and 4 DVE ops (vs 2 ACT + 8 DVE before):

  - conv expressed as a K=2 matmul producing xT [64, batch, time] (bf16)
  - per step, input-side contributions (W @ x_t / W2 @ h1-block, biases
    via an augmented ones-row) are batched BLK timesteps per matmul;
    recurrent parts (U @ h_{t-1}) accumulate on top (bf16 stationary,
    M padded to 128 for FWL).
  - c >= 0 by induction (c = f*c + i*relu(g) from c0=0), so
    h = o*relu(c) = o*c: the final relu is dropped.
  - c pre-zeroed once, so step 0 runs the same code path (f*0 + i*g).
  - h1 ring and h2 double-buffer live in ONE bf16 tile so a single DVE
    op writes both cells' h via a strided 2-slot AP.
"""

import numpy as np

import concourse.bass as bass
import concourse.bacc as bacc
import concourse.mybir as mybir
import concourse.tile as tile
from concourse.bass_utils import run_bass_kernel_spmd

# Problem constants (hardcoded: harness runs kernel.py standalone).
B = 128          # batch
T = 2048         # input sequence length
A = 3            # actions
H = 100          # LSTM units
F = 64           # conv filters
NCORES = 8
BS = B // NCORES  # 16 batch rows per core

GN = 4            # gates
GP = 128          # padded gate size (full 128-col stationary => FWL)
BLK = 8           # timestep block for batched input-side matmuls
RING = 2 * BLK    # h1 ring buffer slots
CCH = 32          # conv time-chunk (N = BS*CCH = 512)
# our gate order [i, f, o, g]; reference weight layout is [i, f, g, o]
GMAP = (0, 1, 3, 2)

f32 = mybir.dt.float32
bf16 = mybir.dt.bfloat16
FT = mybir.ActivationFunctionType
OP = mybir.AluOpType


def build_bass(seq_len=T):
    """Build the single-core program (SPMD: same NEFF on all 8 cores)."""
    TS = seq_len - 1  # conv(kernel=2, VALID) output length
    nc = bacc.Bacc(
        "TRN2",
        target_bir_lowering=False,
        debug=False,
        num_devices=NCORES,
    )

    st_d = nc.dram_tensor("state_input", [BS, seq_len], f32, kind="ExternalInput")
    cw_d = nc.dram_tensor("conv_w", [2, 1, F], f32, kind="ExternalInput")
    cb_d = nc.dram_tensor("conv_b", [F], f32, kind="ExternalInput")
    w1_d = nc.dram_tensor("lstm1_w", [F, GN * H], f32, kind="ExternalInput")
    u1_d = nc.dram_tensor("lstm1_u", [H, GN * H], f32, kind="ExternalInput")
    b1_d = nc.dram_tensor("lstm1_b", [GN * H], f32, kind="ExternalInput")
    w2_d = nc.dram_tensor("lstm2_w", [H, GN * H], f32, kind="ExternalInput")
    u2_d = nc.dram_tensor("lstm2_u", [H, GN * H], f32, kind="ExternalInput")
    b2_d = nc.dram_tensor("lstm2_b", [GN * H], f32, kind="ExternalInput")
    dw_d = nc.dram_tensor("dense_w", [H, A], f32, kind="ExternalInput")
    db_d = nc.dram_tensor("dense_b", [A], f32, kind="ExternalInput")
    out_d = nc.dram_tensor("out", [BS, A], f32, kind="ExternalOutput")

    with tile.TileContext(nc) as tc:
        with (
            tc.tile_pool(name="const", bufs=1) as const,
            tc.tile_pool(name="prep", bufs=2) as prep,
            tc.tile_pool(name="sig", bufs=4) as sigp,
            tc.tile_pool(name="tmp", bufs=4) as tmpp,
            tc.tile_pool(name="zpool", bufs=2, space="PSUM") as zpool,
            tc.tile_pool(name="convpool", bufs=2, space="PSUM") as convpool,
            tc.tile_pool(name="miscpsum", bufs=1, space="PSUM") as miscpsum,
        ):
            # ---------------- input staging ----------------
            s_f32 = prep.tile([BS, seq_len], f32)
            nc.sync.dma_start(out=s_f32, in_=st_d[:, :])
            s_bf = prep.tile([BS, seq_len], bf16)
            nc.vector.tensor_copy(out=s_bf, in_=s_f32)
            # S2[k, b, t] = s[b, t+k]  (conv rhs, contraction dim K=2)
            S2 = const.tile([2, BS, TS], bf16)
            for k in range(2):
                nc.sync.dma_start(out=S2[k : k + 1, :, :], in_=s_bf[:, k : k + TS])

            # xT augmented with a ones-row (bias via matmul)
            xTa = const.tile([F + 1, BS, TS], bf16)
            nc.vector.memset(xTa[F : F + 1, :, :], 1.0)
            # h1 ring (slots 0..RING-1) + h2 double-buffer (slots RING,
            # RING+1), augmented ones-row for W2's bias. Partition ranges
            # must start 32-aligned, so memset [96:101]; rows 96-99 are
            # rewritten with real h values before any consumer reads them.
            hall = const.tile([H + 1, RING + 2, BS], bf16)
            nc.vector.memset(hall[96 : H + 1, :, :], 1.0)

            # ---------------- weights ----------------
            def load_wu(w_dram, b_dram, K, name):
                P = K + (1 if b_dram is not None else 0)
                stage = prep.tile([P, GN * H], f32, tag=f"wstage_{name}")
                if b_dram is not None:
                    # bias row lives at partition K; partition starts must be
                    # 32-aligned, so broadcast into [aligned:K+1] first and
                    # let the weight DMA below overwrite rows [aligned:K).
                    al = (K // 32) * 32
                    bias_bcast = bass.AP(
                        tensor=b_dram[:].tensor,
                        offset=0,
                        ap=[[0, K + 1 - al], [1, GN * H]],
                    )
                    nc.gpsimd.dma_start(out=stage[al : K + 1, :], in_=bias_bcast)
                nc.sync.dma_start(out=stage[0:K, :], in_=w_dram[:, :])
                wt = const.tile([P, GN, GP], bf16, tag=f"wt_{name}")
                for g in range(GN):
                    rg = GMAP[g]
                    nc.vector.tensor_copy(
                        out=wt[:, g, 0:H], in_=stage[:, rg * H : (rg + 1) * H]
                    )
                    nc.vector.memset(wt[:, g, H:GP], 0.0)
                return wt

            U1 = load_wu(u1_d, None, H, "u1")     # [100, 4, 128]
            U2 = load_wu(u2_d, None, H, "u2")     # [100, 4, 128]
            W1b = load_wu(w1_d, b1_d, F, "w1")    # [65, 4, 128]
            W2b = load_wu(w2_d, b2_d, H, "w2")    # [101, 4, 128]

            cwstage = prep.tile([2, F], f32)
            nc.sync.dma_start(out=cwstage, in_=cw_d[:, 0, :])
            cw_bf = const.tile([2, F], bf16)
            nc.vector.tensor_copy(out=cw_bf, in_=cwstage)
            cb_sb = const.tile([F, 1], f32)
            nc.sync.dma_start(out=cb_sb, in_=cb_d[:])

            dw_sb = const.tile([H, A], f32)
            nc.sync.dma_start(out=dw_sb, in_=dw_d[:, :])
            db_sb = const.tile([BS, A], f32)
            db_bcast = bass.AP(
                tensor=db_d[:].tensor, offset=0, ap=[[0, BS], [1, A]]
            )
            nc.gpsimd.dma_start(out=db_sb, in_=db_bcast)

            # ---------------- conv as K=2 matmul ----------------
            for c0 in range(0, TS, CCH):
                cn = min(CCH, TS - c0)
                cp = convpool.tile([F, BS, CCH], f32, tag="convp")
                nc.tensor.matmul(
                    out=cp[:, :, 0:cn],
                    lhsT=cw_bf,
                    rhs=S2[:, :, c0 : c0 + cn],
                    start=True,
                    stop=True,
                )
                nc.scalar.activation(
                    out=xTa[0:F, :, c0 : c0 + cn],
                    in_=cp[:, :, 0:cn],
                    func=FT.Relu,
                    bias=cb_sb,
                    scale=1.0,
                )

            # ---------------- the scan (both cells lock-step) ----------
            # cell1 processes step s; cell2 processes u = s - BLK.
            cc = const.tile([H, 2, BS], f32)      # c state, both cells
            nc.vector.memset(cc[:, :, :], 0.0)
            h2f = const.tile([H, 1, BS], f32)     # final h2 for dense

            zc = None
            for s in range(TS + BLK):
                bi = s % BLK
                u = s - BLK
                c1_on = s < TS
                c2_on = u >= 0
                if bi == 0:
                    zc = zpool.tile([GP, 2, GN, BS, BLK], f32, tag="zc")
                    if c1_on:
                        n = min(BLK, TS - s)
                        rhs = xTa[:, :, s : s + n]
                        for g in range(GN):
                            nc.tensor.matmul(
                                out=zc[:, 0, g, :, 0:n],
                                lhsT=W1b[:, g, :],
                                rhs=rhs,
                                start=True,
                                stop=False,
                                skip_group_check=True,
                            )
                    if c2_on:
                        n2 = min(BLK, TS - u)
                        base = (u // BLK) % 2 * BLK
                        rhs = hall[:, base : base + n2, :].rearrange(
                            "p s b -> p b s"
                        )
                        for g in range(GN):
                            nc.tensor.matmul(
                                out=zc[:, 1, g, :, 0:n2],
                                lhsT=W2b[:, g, :],
                                rhs=rhs,
                                start=True,
                                stop=False,
                                skip_group_check=True,
                            )
                if c1_on and s > 0:
                    rhs1 = hall[0:H, (s - 1) % RING, :]
                    for g in range(GN):
                        nc.tensor.matmul(
                            out=zc[:, 0, g, :, bi],
                            lhsT=U1[:, g, :],
                            rhs=rhs1,
                            start=False,
                            stop=True,
                            skip_group_check=True,
                        )
                if c2_on and u > 0:
                    rhs2 = hall[0:H, RING + (u - 1) % 2, :]
                    for g in range(GN):
                        nc.tensor.matmul(
                            out=zc[:, 1, g, :, bi],
                            lhsT=U2[:, g, :],
                            rhs=rhs2,
                            start=False,
                            stop=True,
                            skip_group_check=True,
                        )

                # -------- elementwise: one ACT + 4 DVE for both cells ----
                lo, hi = (0 if c1_on else 1), (2 if c2_on else 1)
                cs = slice(lo, hi)
                sg = sigp.tile([H, 2, 3, BS], f32, tag="sg")
                nc.scalar.activation(
                    out=sg[:, cs, :, :],
                    in_=zc[0:H, cs, 0:3, :, bi],
                    func=FT.Sigmoid,
                )
                t1 = tmpp.tile([H, 2, BS], f32, tag="t1")
                # t1 = relu(g) * i
                nc.vector.scalar_tensor_tensor(
                    out=t1[:, cs, :], in0=zc[0:H, cs, 3, :, bi], scalar=0.0,
                    in1=sg[:, cs, 0, :], op0=OP.max, op1=OP.mult,
                )
                t2 = tmpp.tile([H, 2, BS], f32, tag="t2")
                nc.vector.tensor_mul(
                    out=t2[:, cs, :], in0=sg[:, cs, 1, :], in1=cc[:, cs, :]
                )
                nc.vector.tensor_add(
                    out=cc[:, cs, :], in0=t1[:, cs, :], in1=t2[:, cs, :]
                )
                # h = o * c  (c >= 0 always, so relu(c) == c)
                if c1_on and c2_on:
                    a, b = s % RING, RING + u % 2
                    hout = hall[0:H, a : b + 1 : b - a, :]
                elif c1_on:
                    hout = hall[0:H, s % RING : s % RING + 1, :]
                elif u < TS - 1:
                    hout = hall[0:H, RING + u % 2 : RING + u % 2 + 1, :]
                else:  # final step: only h2 remains, straight to f32
                    hout = h2f[:, :, :]
                nc.vector.tensor_mul(
                    out=hout, in0=sg[:, cs, 2, :], in1=cc[:, cs, :]
                )

            # ---------------- dense + softmax ----------------
            lg_ps = miscpsum.tile([BS, A], f32)
            nc.tensor.matmul(
                out=lg_ps, lhsT=h2f[:, 0, :], rhs=dw_sb, start=True, stop=True
            )
            lg = tmpp.tile([BS, A], f32, tag="lg")
            nc.vector.tensor_add(out=lg, in0=lg_ps, in1=db_sb)
            mx = tmpp.tile([BS, 1], f32, tag="mx")
            nc.vector.tensor_reduce(
                out=mx, in_=lg, axis=mybir.AxisListType.X, op=OP.max
            )
            nmx = tmpp.tile([BS, 1], f32, tag="nmx")
            nc.vector.tensor_scalar_mul(out=nmx, in0=mx, scalar1=-1.0)
            ex = tmpp.tile([BS, A], f32, tag="ex")
            nc.scalar.activation(out=ex, in_=lg, func=FT.Exp, bias=nmx, scale=1.0)
            sm = tmpp.tile([BS, 1], f32, tag="sm")
            nc.vector.tensor_reduce(
                out=sm, in_=ex, axis=mybir.AxisListType.X, op=OP.add
            )
            rc = tmpp.tile([BS, 1], f32, tag="rc")
            nc.vector.reciprocal(out=rc, in_=sm)
            ot = tmpp.tile([BS, A], f32, tag="ot")
            nc.vector.tensor_scalar_mul(out=ot, in0=ex, scalar1=rc)
            nc.sync.dma_start(out=out_d[:, :], in_=ot)

    nc.finalize()
    return nc


_NC_CACHE = {}


def _get_nc(seq_len=T):
    if seq_len not in _NC_CACHE:
        _NC_CACHE[seq_len] = build_bass(seq_len)
    return _NC_CACHE[seq_len]


def kernel(**inputs):
    return run(inputs, seq_len=T, trace=False)[0]


def run(inputs, seq_len=T, trace=False):
    """Returns (full_output [B, A] f32, BassKernelResults)."""
    nc = _get_nc(seq_len)
    state = np.ascontiguousarray(
        np.asarray(inputs["state_input"], dtype=np.float32).reshape(B, seq_len)
    )
    shared = {}
    for k in (
        "conv_w", "conv_b", "lstm1_w", "lstm1_u", "lstm1_b",
        "lstm2_w", "lstm2_u", "lstm2_b", "dense_w", "dense_b",
    ):
        shared[k] = np.ascontiguousarray(np.asarray(inputs[k], dtype=np.float32))
    in_maps = []
    for c in range(NCORES):
        m = dict(shared)
        m["state_input"] = np.ascontiguousarray(state[c * BS : (c + 1) * BS])
        in_maps.append(m)
    res = run_bass_kernel_spmd(
        nc, in_maps, core_ids=list(range(NCORES)), trace=trace
    )
    out = np.concatenate([r["out"] for r in res.results], axis=0)
    return out.astype(np.float32), res


# revision 5
# speedup vs baseline: 1.2781x; 1.0497x over previous
"""Conv1D + 2x LSTM(relu) + dense/softmax actor model on 8 Trainium2 cores.

Strategy: pure data parallel over batch (128 -> 16 per core); params
replicated. Everything kept on-chip in a "transposed" layout
([units on partitions, batch on free]) so the sequential LSTM recurrence
never needs an on-chip transpose.

v2: the two LSTM cells are processed in lock-step (cell2 lags cell1 by
BLK steps) with their gate pre-activations in ONE shared PSUM tile
[128, 2cells, 4gates, BS, BLK], so each step needs a single sigmoid ACT
(i,f,o for both cells) and 4 DVE ops (vs 2 ACT + 8 DVE before):

  - conv expressed as a K=2 matmul producing xT [64, batch, time] (bf16)
  - per step, input-side contributions (W @ x_t / W2 @ h1-block, biases
    via an augmented ones-row) are batched BLK timesteps per matmul;
    recurrent parts (U @ h_{t-1}) accumulate on top (bf16 stationary,
    M padded to 128 for FWL).
  - c >= 0 by induction (c = f*c + i*relu(g) from c0=0), so
    h = o*relu(c) = o*c: the final relu is dropped.
  - c pre-zeroed once, so step 0 runs the same code path (f*0 + i*g).
  - h1 ring and h2 double-buffer live in ONE bf16 tile so a single DVE
    op writes both cells' h via a strided 2-slot AP.
"""

import numpy as np

import concourse.bass as bass
import concourse.bacc as bacc
import concourse.mybir as mybir
import concourse.tile as tile
from concourse.bass_utils import run_bass_kernel_spmd

# Problem constants (hardcoded: harness runs kernel.py standalone).
B = 128          # batch
T = 2048         # input sequence length
A = 3            # actions
H = 100          # LSTM units
F = 64           # conv filters
NCORES = 8
BS = B // NCORES  # 16 batch rows per core

GN = 4            # gates
GP = 128          # padded gate size (full 128-col stationary => FWL)
BLK = 16          # timestep block for batched input-side matmuls
RING = 2 * BLK    # h1 ring buffer slots
CCH = 32          # conv time-chunk (N = BS*CCH = 512)
# our gate order [i, f, o, g]; reference weight layout is [i, f, g, o]
GMAP = (0, 1, 3, 2)

f32 = mybir.dt.float32
bf16 = mybir.dt.bfloat16
FT = mybir.ActivationFunctionType
OP = mybir.AluOpType


def build_bass(seq_len=T):
    """Build the single-core program (SPMD: same NEFF on all 8 cores)."""
    TS = seq_len - 1  # conv(kernel=2, VALID) output length
    nc = bacc.Bacc(
        "TRN2",
        target_bir_lowering=False,
        debug=False,
        num_devices=NCORES,
    )

    st_d = nc.dram_tensor("state_input", [BS, seq_len], f32, kind="ExternalInput")
    cw_d = nc.dram_tensor("conv_w", [2, 1, F], f32, kind="ExternalInput")
    cb_d = nc.dram_tensor("conv_b", [F], f32, kind="ExternalInput")
    w1_d = nc.dram_tensor("lstm1_w", [F, GN * H], f32, kind="ExternalInput")
    u1_d = nc.dram_tensor("lstm1_u", [H, GN * H], f32, kind="ExternalInput")
    b1_d = nc.dram_tensor("lstm1_b", [GN * H], f32, kind="ExternalInput")
    w2_d = nc.dram_tensor("lstm2_w", [H, GN * H], f32, kind="ExternalInput")
    u2_d = nc.dram_tensor("lstm2_u", [H, GN * H], f32, kind="ExternalInput")
    b2_d = nc.dram_tensor("lstm2_b", [GN * H], f32, kind="ExternalInput")
    dw_d = nc.dram_tensor("dense_w", [H, A], f32, kind="ExternalInput")
    db_d = nc.dram_tensor("dense_b", [A], f32, kind="ExternalInput")
    out_d = nc.dram_tensor("out", [BS, A], f32, kind="ExternalOutput")

    with tile.TileContext(nc) as tc:
        with (
            tc.tile_pool(name="const", bufs=1) as const,
            tc.tile_pool(name="prep", bufs=2) as prep,
            tc.tile_pool(name="sig", bufs=4) as sigp,
            tc.tile_pool(name="tmp", bufs=4) as tmpp,
            tc.tile_pool(name="zpool", bufs=1, space="PSUM") as zpool,
            tc.tile_pool(name="convpool", bufs=2, space="PSUM") as convpool,
            tc.tile_pool(name="miscpsum", bufs=1, space="PSUM") as miscpsum,
        ):
            # ---------------- input staging ----------------
            s_f32 = prep.tile([BS, seq_len], f32)
            nc.sync.dma_start(out=s_f32, in_=st_d[:, :])
            s_bf = prep.tile([BS, seq_len], bf16)
            nc.vector.tensor_copy(out=s_bf, in_=s_f32)
            # S2[k, b, t] = s[b, t+k]  (conv rhs, contraction dim K=2)
            S2 = const.tile([2, BS, TS], bf16)
            for k in range(2):
                nc.sync.dma_start(out=S2[k : k + 1, :, :], in_=s_bf[:, k : k + TS])

            # xT augmented with a ones-row (bias via matmul)
            xTa = const.tile([F + 1, BS, TS], bf16)
            nc.vector.memset(xTa[F : F + 1, :, :], 1.0)
            # h1 ring (slots 0..RING-1) + h2 double-buffer (slots RING,
            # RING+1), augmented ones-row for W2's bias. Partition ranges
            # must start 32-aligned, so memset [96:101]; rows 96-99 are
            # rewritten with real h values before any consumer reads them.
            hall = const.tile([H + 1, RING + 2, BS], bf16)
            nc.vector.memset(hall[96 : H + 1, :, :], 1.0)

            # ---------------- weights ----------------
            def load_wu(w_dram, b_dram, K, name):
                P = K + (1 if b_dram is not None else 0)
                stage = prep.tile([P, GN * H], f32, tag=f"wstage_{name}")
                if b_dram is not None:
                    # bias row lives at partition K; partition starts must be
                    # 32-aligned, so broadcast into [aligned:K+1] first and
                    # let the weight DMA below overwrite rows [aligned:K).
                    al = (K // 32) * 32
                    bias_bcast = bass.AP(
                        tensor=b_dram[:].tensor,
                        offset=0,
                        ap=[[0, K + 1 - al], [1, GN * H]],
                    )
                    nc.gpsimd.dma_start(out=stage[al : K + 1, :], in_=bias_bcast)
                nc.sync.dma_start(out=stage[0:K, :], in_=w_dram[:, :])
                wt = const.tile([P, GN, GP], bf16, tag=f"wt_{name}")
                for g in range(GN):
                    rg = GMAP[g]
                    nc.vector.tensor_copy(
                        out=wt[:, g, 0:H], in_=stage[:, rg * H : (rg + 1) * H]
                    )
                    nc.vector.memset(wt[:, g, H:GP], 0.0)
                return wt

            U1 = load_wu(u1_d, None, H, "u1")     # [100, 4, 128]
            U2 = load_wu(u2_d, None, H, "u2")     # [100, 4, 128]
            W1b = load_wu(w1_d, b1_d, F, "w1")    # [65, 4, 128]
            W2b = load_wu(w2_d, b2_d, H, "w2")    # [101, 4, 128]

            cwstage = prep.tile([2, F], f32)
            nc.sync.dma_start(out=cwstage, in_=cw_d[:, 0, :])
            cw_bf = const.tile([2, F], bf16)
            nc.vector.tensor_copy(out=cw_bf, in_=cwstage)
            cb_sb = const.tile([F, 1], f32)
            nc.sync.dma_start(out=cb_sb, in_=cb_d[:])

            dw_sb = const.tile([H, A], f32)
            nc.sync.dma_start(out=dw_sb, in_=dw_d[:, :])
            db_sb = const.tile([BS, A], f32)
            db_bcast = bass.AP(
                tensor=db_d[:].tensor, offset=0, ap=[[0, BS], [1, A]]
            )
            nc.gpsimd.dma_start(out=db_sb, in_=db_bcast)

            # ---------------- conv as K=2 matmul ----------------
            for c0 in range(0, TS, CCH):
                cn = min(CCH, TS - c0)
                cp = convpool.tile([F, BS, CCH], f32, tag="convp")
                nc.tensor.matmul(
                    out=cp[:, :, 0:cn],
                    lhsT=cw_bf,
                    rhs=S2[:, :, c0 : c0 + cn],
                    start=True,
                    stop=True,
                )
                nc.scalar.activation(
                    out=xTa[0:F, :, c0 : c0 + cn],
                    in_=cp[:, :, 0:cn],
                    func=FT.Relu,
                    bias=cb_sb,
                    scale=1.0,
                )

            # ---------------- the scan (both cells lock-step) ----------
            # cell1 processes step s; cell2 processes u = s - BLK.
            cc = const.tile([H, 2, BS], f32)      # c state, both cells
            nc.vector.memset(cc[:, :, :], 0.0)
            h2f = const.tile([H, 1, BS], f32)     # final h2 for dense

            zc = None
            for s in range(TS + BLK):
                bi = s % BLK
                u = s - BLK
                c1_on = s < TS
                c2_on = u >= 0
                if bi == 0:
                    zc = zpool.tile([GP, 2, GN, BS, BLK], f32, tag="zc")
                    if c1_on:
                        n = min(BLK, TS - s)
                        rhs = xTa[:, :, s : s + n]
                        for g in range(GN):
                            nc.tensor.matmul(
                                out=zc[:, 0, g, :, 0:n],
                                lhsT=W1b[:, g, :],
                                rhs=rhs,
                                start=True,
                                stop=False,
                                skip_group_check=True,
                            )
                    if c2_on:
                        n2 = min(BLK, TS - u)
                        base = (u // BLK) % 2 * BLK
                        rhs = hall[:, base : base + n2, :].rearrange(
                            "p s b -> p b s"
                        )
                        for g in range(GN):
                            nc.tensor.matmul(
                                out=zc[:, 1, g, :, 0:n2],
                                lhsT=W2b[:, g, :],
                                rhs=rhs,
                                start=True,
                                stop=False,
                                skip_group_check=True,
                            )
                if c1_on and s > 0:
                    rhs1 = hall[0:H, (s - 1) % RING, :]
                    for g in range(GN):
                        nc.tensor.matmul(
                            out=zc[:, 0, g, :, bi],
                            lhsT=U1[:, g, :],
                            rhs=rhs1,
                            start=False,
                            stop=True,
                            skip_group_check=True,
                        )
                if c2_on and u > 0:
                    rhs2 = hall[0:H, RING + (u - 1) % 2, :]
                    for g in range(GN):
                        nc.tensor.matmul(
                            out=zc[:, 1, g, :, bi],
                            lhsT=U2[:, g, :],
                            rhs=rhs2,
                            start=False,
                            stop=True,
                            skip_group_check=True,
                        )

                # -------- elementwise: one ACT + 4 DVE for both cells ----
                lo, hi = (0 if c1_on else 1), (2 if c2_on else 1)
                cs = slice(lo, hi)
                sg = sigp.tile([H, 2, 3, BS], f32, tag="sg")
                nc.scalar.activation(
                    out=sg[:, cs, :, :],
                    in_=zc[0:H, cs, 0:3, :, bi],
                    func=FT.Sigmoid,
                )
                t1 = tmpp.tile([H, 2, BS], f32, tag="t1")
                # t1 = relu(g) * i
                nc.vector.scalar_tensor_tensor(
                    out=t1[:, cs, :], in0=zc[0:H, cs, 3, :, bi], scalar=0.0,
                    in1=sg[:, cs, 0, :], op0=OP.max, op1=OP.mult,
                )
                t2 = tmpp.tile([H, 2, BS], f32, tag="t2")
                nc.vector.tensor_mul(
                    out=t2[:, cs, :], in0=sg[:, cs, 1, :], in1=cc[:, cs, :]
                )
                nc.vector.tensor_add(
                    out=cc[:, cs, :], in0=t1[:, cs, :], in1=t2[:, cs, :]
                )
                # h = o * c  (c >= 0 always, so relu(c) == c)
                if c1_on and c2_on:
                    a, b = s % RING, RING + u % 2
                    hout = hall[0:H, a : b + 1 : b - a, :]
                elif c1_on:
                    hout = hall[0:H, s % RING : s % RING + 1, :]
                elif u < TS - 1:
                    hout = hall[0:H, RING + u % 2 : RING + u % 2 + 1, :]
                else:  # final step: only h2 remains, straight to f32
                    hout = h2f[:, :, :]
                nc.vector.tensor_mul(
                    out=hout, in0=sg[:, cs, 2, :], in1=cc[:, cs, :]
                )

            # ---------------- dense + softmax ----------------
            lg_ps = miscpsum.tile([BS, A], f32)
            nc.tensor.matmul(
                out=lg_ps, lhsT=h2f[:, 0, :], rhs=dw_sb, start=True, stop=True
            )
            lg = tmpp.tile([BS, A], f32, tag="lg")
            nc.vector.tensor_add(out=lg, in0=lg_ps, in1=db_sb)
            mx = tmpp.tile([BS, 1], f32, tag="mx")
            nc.vector.tensor_reduce(
                out=mx, in_=lg, axis=mybir.AxisListType.X, op=OP.max
            )
            nmx = tmpp.tile([BS, 1], f32, tag="nmx")
            nc.vector.tensor_scalar_mul(out=nmx, in0=mx, scalar1=-1.0)
            ex = tmpp.tile([BS, A], f32, tag="ex")
            nc.scalar.activation(out=ex, in_=lg, func=FT.Exp, bias=nmx, scale=1.0)
            sm = tmpp.tile([BS, 1], f32, tag="sm")
            nc.vector.tensor_reduce(
                out=sm, in_=ex, axis=mybir.AxisListType.X, op=OP.add
            )
            rc = tmpp.tile([BS, 1], f32, tag="rc")
            nc.vector.reciprocal(out=rc, in_=sm)
            ot = tmpp.tile([BS, A], f32, tag="ot")
            nc.vector.tensor_scalar_mul(out=ot, in0=ex, scalar1=rc)
            nc.sync.dma_start(out=out_d[:, :], in_=ot)

    nc.finalize()
    return nc


_NC_CACHE = {}


def _get_nc(seq_len=T):
    if seq_len not in _NC_CACHE:
        _NC_CACHE[seq_len] = build_bass(seq_len)
    return _NC_CACHE[seq_len]


def kernel(**inputs):
    return run(inputs, seq_len=T, trace=False)[0]


def run(inputs, seq_len=T, trace=False):
    """Returns (full_output [B, A] f32, BassKernelResults)."""
    nc = _get_nc(seq_len)
    state = np.ascontiguousarray(
        np.asarray(inputs["state_input"], dtype=np.float32).reshape(B, seq_len)
    )
    shared = {}
    for k in (
        "conv_w", "conv_b", "lstm1_w", "lstm1_u", "lstm1_b",
        "lstm2_w", "lstm2_u", "lstm2_b", "dense_w", "dense_b",
    ):
        shared[k] = np.ascontiguousarray(np.asarray(inputs[k], dtype=np.float32))
    in_maps = []
    for c in range(NCORES):
        m = dict(shared)
        m["state_input"] = np.ascontiguousarray(state[c * BS : (c + 1) * BS])
        in_maps.append(m)
    res = run_bass_kernel_spmd(
        nc, in_maps, core_ids=list(range(NCORES)), trace=trace
    )
    out = np.concatenate([r["out"] for r in res.results], axis=0)
    return out.astype(np.float32), res


# revision 9
# speedup vs baseline: 1.2917x; 1.0106x over previous
"""Conv1D + 2x LSTM(relu) + dense/softmax actor model on 8 Trainium2 cores.

Strategy: pure data parallel over batch (128 -> 16 per core); params
replicated. Everything kept on-chip in a "transposed" layout
([units on partitions, batch on free]) so the sequential LSTM recurrence
never needs an on-chip transpose.

v2: the two LSTM cells are processed in lock-step (cell2 lags cell1 by
BLK steps) with their gate pre-activations in ONE shared PSUM tile
[128, 2cells, 4gates, BS, BLK], so each step needs a single sigmoid ACT
(i,f,o for both cells) and 4 DVE ops (vs 2 ACT + 8 DVE before):

  - conv expressed as a K=2 matmul producing xT [64, batch, time] (bf16)
  - per step, input-side contributions (W @ x_t / W2 @ h1-block, biases
    via an augmented ones-row) are batched BLK timesteps per matmul;
    recurrent parts (U @ h_{t-1}) accumulate on top (bf16 stationary,
    M padded to 128 for FWL).
  - c >= 0 by induction (c = f*c + i*relu(g) from c0=0), so
    h = o*relu(c) = o*c: the final relu is dropped.
  - c pre-zeroed once, so step 0 runs the same code path (f*0 + i*g).
  - h1 ring and h2 double-buffer live in ONE bf16 tile so a single DVE
    op writes both cells' h via a strided 2-slot AP.
"""

import numpy as np

import concourse.bass as bass
import concourse.bacc as bacc
import concourse.mybir as mybir
import concourse.tile as tile
from concourse.bass_utils import run_bass_kernel_spmd

# Problem constants (hardcoded: harness runs kernel.py standalone).
B = 128          # batch
T = 2048         # input sequence length
A = 3            # actions
H = 100          # LSTM units
F = 64           # conv filters
NCORES = 8
BS = B // NCORES  # 16 batch rows per core

GN = 4            # gates
GP = 128          # padded gate size (full 128-col stationary => FWL)
BLK = 16          # timestep block for batched input-side matmuls
RING = 2 * BLK    # h1 ring buffer slots
CCH = 32          # conv time-chunk (N = BS*CCH = 512)
# our gate order [i, f, o, g]; reference weight layout is [i, f, g, o]
GMAP = (0, 1, 3, 2)

f32 = mybir.dt.float32
bf16 = mybir.dt.bfloat16
FT = mybir.ActivationFunctionType
OP = mybir.AluOpType


def build_bass(seq_len=T):
    """Build the single-core program (SPMD: same NEFF on all 8 cores)."""
    TS = seq_len - 1  # conv(kernel=2, VALID) output length
    nc = bacc.Bacc(
        "TRN2",
        target_bir_lowering=False,
        debug=False,
        num_devices=NCORES,
    )

    st_d = nc.dram_tensor("state_input", [BS, seq_len], f32, kind="ExternalInput")
    cw_d = nc.dram_tensor("conv_w", [2, 1, F], f32, kind="ExternalInput")
    cb_d = nc.dram_tensor("conv_b", [F], f32, kind="ExternalInput")
    w1_d = nc.dram_tensor("lstm1_w", [F, GN * H], f32, kind="ExternalInput")
    u1_d = nc.dram_tensor("lstm1_u", [H, GN * H], f32, kind="ExternalInput")
    b1_d = nc.dram_tensor("lstm1_b", [GN * H], f32, kind="ExternalInput")
    w2_d = nc.dram_tensor("lstm2_w", [H, GN * H], f32, kind="ExternalInput")
    u2_d = nc.dram_tensor("lstm2_u", [H, GN * H], f32, kind="ExternalInput")
    b2_d = nc.dram_tensor("lstm2_b", [GN * H], f32, kind="ExternalInput")
    dw_d = nc.dram_tensor("dense_w", [H, A], f32, kind="ExternalInput")
    db_d = nc.dram_tensor("dense_b", [A], f32, kind="ExternalInput")
    out_d = nc.dram_tensor("out", [BS, A], f32, kind="ExternalOutput")

    with tile.TileContext(nc) as tc:
        with (
            tc.tile_pool(name="const", bufs=1) as const,
            tc.tile_pool(name="prep", bufs=2) as prep,
            tc.tile_pool(name="sig", bufs=4) as sigp,
            tc.tile_pool(name="tmp", bufs=4) as tmpp,
            tc.tile_pool(name="zpool", bufs=1, space="PSUM") as zpool,
            tc.tile_pool(name="convpool", bufs=2, space="PSUM") as convpool,
            tc.tile_pool(name="miscpsum", bufs=1, space="PSUM") as miscpsum,
        ):
            # ---------------- input staging ----------------
            s_f32 = prep.tile([BS, seq_len], f32)
            nc.sync.dma_start(out=s_f32, in_=st_d[:, :])
            s_bf = prep.tile([BS, seq_len], bf16)
            nc.vector.tensor_copy(out=s_bf, in_=s_f32)
            # S2[k, b, t] = s[b, t+k]  (conv rhs, contraction dim K=2)
            S2 = const.tile([2, BS, TS], bf16)
            for k in range(2):
                nc.sync.dma_start(out=S2[k : k + 1, :, :], in_=s_bf[:, k : k + TS])

            # xT augmented with a ones-row (bias via matmul)
            xTa = const.tile([F + 1, BS, TS], bf16)
            nc.vector.memset(xTa[F : F + 1, :, :], 1.0)
            # h1 ring (slots 0..RING-1) + h2 double-buffer (slots RING,
            # RING+1), augmented ones-row for W2's bias. Partition ranges
            # must start 32-aligned, so memset [96:101]; rows 96-99 are
            # rewritten with real h values before any consumer reads them.
            hall = const.tile([H + 1, RING + 2, BS], bf16)
            nc.vector.memset(hall[96 : H + 1, :, :], 1.0)

            # ---------------- weights ----------------
            def load_wu(w_dram, b_dram, K, name):
                P = K + (1 if b_dram is not None else 0)
                stage = prep.tile([P, GN * H], f32, tag=f"wstage_{name}")
                if b_dram is not None:
                    # bias row lives at partition K; partition starts must be
                    # 32-aligned, so broadcast into [aligned:K+1] first and
                    # let the weight DMA below overwrite rows [aligned:K).
                    al = (K // 32) * 32
                    bias_bcast = bass.AP(
                        tensor=b_dram[:].tensor,
                        offset=0,
                        ap=[[0, K + 1 - al], [1, GN * H]],
                    )
                    nc.gpsimd.dma_start(out=stage[al : K + 1, :], in_=bias_bcast)
                nc.sync.dma_start(out=stage[0:K, :], in_=w_dram[:, :])
                wt = const.tile([P, GN, GP], bf16, tag=f"wt_{name}")
                for g in range(GN):
                    rg = GMAP[g]
                    nc.vector.tensor_copy(
                        out=wt[:, g, 0:H], in_=stage[:, rg * H : (rg + 1) * H]
                    )
                    nc.vector.memset(wt[:, g, H:GP], 0.0)
                return wt

            U1 = load_wu(u1_d, None, H, "u1")     # [100, 4, 128]
            U2 = load_wu(u2_d, None, H, "u2")     # [100, 4, 128]
            W1b = load_wu(w1_d, b1_d, F, "w1")    # [65, 4, 128]
            W2b = load_wu(w2_d, b2_d, H, "w2")    # [101, 4, 128]

            cwstage = prep.tile([2, F], f32)
            nc.sync.dma_start(out=cwstage, in_=cw_d[:, 0, :])
            cw_bf = const.tile([2, F], bf16)
            nc.vector.tensor_copy(out=cw_bf, in_=cwstage)
            cb_sb = const.tile([F, 1], f32)
            nc.sync.dma_start(out=cb_sb, in_=cb_d[:])

            dw_sb = const.tile([H, A], f32)
            nc.sync.dma_start(out=dw_sb, in_=dw_d[:, :])
            db_sb = const.tile([BS, A], f32)
            db_bcast = bass.AP(
                tensor=db_d[:].tensor, offset=0, ap=[[0, BS], [1, A]]
            )
            nc.gpsimd.dma_start(out=db_sb, in_=db_bcast)

            # ---------------- conv as K=2 matmul ----------------
            for c0 in range(0, TS, CCH):
                cn = min(CCH, TS - c0)
                cp = convpool.tile([F, BS, CCH], f32, tag="convp")
                nc.tensor.matmul(
                    out=cp[:, :, 0:cn],
                    lhsT=cw_bf,
                    rhs=S2[:, :, c0 : c0 + cn],
                    start=True,
                    stop=True,
                )
                nc.scalar.activation(
                    out=xTa[0:F, :, c0 : c0 + cn],
                    in_=cp[:, :, 0:cn],
                    func=FT.Relu,
                    bias=cb_sb,
                    scale=1.0,
                )

            # ---------------- the scan (both cells lock-step) ----------
            # cell1 processes step s; cell2 processes u = s - BLK.
            cc = const.tile([H, 2, BS], f32)      # c state, both cells
            nc.vector.memset(cc[:, :, :], 0.0)
            h2f = const.tile([H, 1, BS], f32)     # final h2 for dense

            zc = None
            for s in range(TS + BLK):
                bi = s % BLK
                u = s - BLK
                c1_on = s < TS
                c2_on = u >= 0
                if bi == 0:
                    zc = zpool.tile([GP, 2, GN, BS, BLK], f32, tag="zc")
                    if c1_on:
                        n = min(BLK, TS - s)
                        rhs = xTa[:, :, s : s + n]
                        for g in range(GN):
                            nc.tensor.matmul(
                                out=zc[:, 0, g, :, 0:n],
                                lhsT=W1b[:, g, :],
                                rhs=rhs,
                                start=True,
                                stop=False,
                                skip_group_check=True,
                            )
                    if c2_on:
                        n2 = min(BLK, TS - u)
                        base = (u // BLK) % 2 * BLK
                        rhs = hall[:, base : base + n2, :].rearrange(
                            "p s b -> p b s"
                        )
                        for g in range(GN):
                            nc.tensor.matmul(
                                out=zc[:, 1, g, :, 0:n2],
                                lhsT=W2b[:, g, :],
                                rhs=rhs,
                                start=True,
                                stop=False,
                                skip_group_check=True,
                            )
                if c1_on and s > 0:
                    rhs1 = hall[0:H, (s - 1) % RING, :]
                    for g in range(GN):
                        nc.tensor.matmul(
                            out=zc[:, 0, g, :, bi],
                            lhsT=U1[:, g, :],
                            rhs=rhs1,
                            start=False,
                            stop=True,
                            skip_group_check=True,
                        )
                if c2_on and u > 0:
                    rhs2 = hall[0:H, RING + (u - 1) % 2, :]
                    for g in range(GN):
                        nc.tensor.matmul(
                            out=zc[:, 1, g, :, bi],
                            lhsT=U2[:, g, :],
                            rhs=rhs2,
                            start=False,
                            stop=True,
                            skip_group_check=True,
                        )

                # -------- elementwise: one ACT + 4 DVE for both cells ----
                lo, hi = (0 if c1_on else 1), (2 if c2_on else 1)
                cs = slice(lo, hi)
                sg = sigp.tile([H, 2, 3, BS], f32, tag="sg")
                nc.scalar.activation(
                    out=sg[:, cs, :, :],
                    in_=zc[0:H, cs, 0:3, :, bi],
                    func=FT.Sigmoid,
                )
                t1 = tmpp.tile([H, 2, BS], f32, tag="t1")
                # t1 = relu(g) * i
                nc.vector.scalar_tensor_tensor(
                    out=t1[:, cs, :], in0=zc[0:H, cs, 3, :, bi], scalar=0.0,
                    in1=sg[:, cs, 0, :], op0=OP.max, op1=OP.mult,
                )
                t2 = tmpp.tile([H, 2, BS], f32, tag="t2")
                nc.vector.tensor_mul(
                    out=t2[:, cs, :], in0=sg[:, cs, 1, :], in1=cc[:, cs, :]
                )
                nc.vector.tensor_add(
                    out=cc[:, cs, :], in0=t1[:, cs, :], in1=t2[:, cs, :]
                )
                # h = o * c  (c >= 0 always, so relu(c) == c)
                if c1_on and c2_on:
                    a, b = s % RING, RING + u % 2
                    hout = hall[0:H, a : b + 1 : b - a, :]
                elif c1_on:
                    hout = hall[0:H, s % RING : s % RING + 1, :]
                elif u < TS - 1:
                    hout = hall[0:H, RING + u % 2 : RING + u % 2 + 1, :]
                else:  # final step: only h2 remains, straight to f32
                    hout = h2f[:, :, :]
                nc.vector.tensor_mul(
                    out=hout, in0=sg[:, cs, 2, :], in1=cc[:, cs, :]
                )

            # ---------------- dense + softmax ----------------
            lg_ps = miscpsum.tile([BS, A], f32)
            nc.tensor.matmul(
                out=lg_ps, lhsT=h2f[:, 0, :], rhs=dw_sb, start=True, stop=True
            )
            lg = tmpp.tile([BS, A], f32, tag="lg")
            nc.vector.tensor_add(out=lg, in0=lg_ps, in1=db_sb)
            mx = tmpp.tile([BS, 1], f32, tag="mx")
            nc.vector.tensor_reduce(
                out=mx, in_=lg, axis=mybir.AxisListType.X, op=OP.max
            )
            nmx = tmpp.tile([BS, 1], f32, tag="nmx")
            nc.vector.tensor_scalar_mul(out=nmx, in0=mx, scalar1=-1.0)
            ex = tmpp.tile([BS, A], f32, tag="ex")
            nc.scalar.activation(out=ex, in_=lg, func=FT.Exp, bias=nmx, scale=1.0)
            sm = tmpp.tile([BS, 1], f32, tag="sm")
            nc.vector.tensor_reduce(
                out=sm, in_=ex, axis=mybir.AxisListType.X, op=OP.add
            )
            rc = tmpp.tile([BS, 1], f32, tag="rc")
            nc.vector.reciprocal(out=rc, in_=sm)
            ot = tmpp.tile([BS, A], f32, tag="ot")
            nc.vector.tensor_scalar_mul(out=ot, in0=ex, scalar1=rc)
            nc.sync.dma_start(out=out_d[:, :], in_=ot)

    nc.finalize()
    return nc


_NC_CACHE = {}


def _get_nc(seq_len=T):
    if seq_len not in _NC_CACHE:
        _NC_CACHE[seq_len] = build_bass(seq_len)
    return _NC_CACHE[seq_len]


def kernel(**inputs):
    return run(inputs, seq_len=T, trace=False)[0]


def run(inputs, seq_len=T, trace=False):
    """Returns (full_output [B, A] f32, BassKernelResults)."""
    nc = _get_nc(seq_len)
    state = np.ascontiguousarray(
        np.asarray(inputs["state_input"], dtype=np.float32).reshape(B, seq_len)
    )
    shared = {}
    for k in (
        "conv_w", "conv_b", "lstm1_w", "lstm1_u", "lstm1_b",
        "lstm2_w", "lstm2_u", "lstm2_b", "dense_w", "dense_b",
    ):
        shared[k] = np.ascontiguousarray(np.asarray(inputs[k], dtype=np.float32))
    in_maps = []
    for c in range(NCORES):
        m = dict(shared)
        m["state_input"] = np.ascontiguousarray(state[c * BS : (c + 1) * BS])
        in_maps.append(m)
    res = run_bass_kernel_spmd(
        nc, in_maps, core_ids=list(range(NCORES)), trace=trace
    )
    out = np.concatenate([r["out"] for r in res.results], axis=0)
    return out.astype(np.float32), res
